# revision 1
# baseline (speedup 1.0000x reference)
"""DualPathLF kernel for 8 Trainium2 NeuronCores.

Sharding: data-parallel over B (B=8 == n_cores). Each core gets one batch
element; both dual-path attentions reshape batch into B*T / B*F so the whole
computation is embarrassingly parallel over B. Params are replicated.

Self-contained: hardcodes shapes B=8, D=64, T=2000, NF=32, H=4.
"""

import numpy as np
import jax
import jax.numpy as jnp

B, D, T, NF, H = 8, 64, 2000, 32, 4
EPS = 1e-5

_PARAM_ORDER = [
    "norm_w", "norm_b",
    "f_ln1_w", "f_ln1_b", "f_Wq", "f_Wk", "f_Wv", "f_Wo", "f_bo",
    "f_ln2_w", "f_ln2_b", "f_W1", "f_b1", "f_W2", "f_b2", "f_Wd", "f_bd",
    "t_ln1_w", "t_ln1_b", "t_Wq", "t_Wk", "t_Wv", "t_Wo", "t_bo",
    "t_ln2_w", "t_ln2_b", "t_W1", "t_b1", "t_W2", "t_b2", "t_Wd", "t_bd",
]


def _ln(x, w, b):
    m = x.mean(-1, keepdims=True)
    v = ((x - m) ** 2).mean(-1, keepdims=True)
    return (x - m) / jnp.sqrt(v + EPS) * w + b


def _ln2d(x, w, b):
    m = x.mean((-2, -1), keepdims=True)
    v = ((x - m) ** 2).mean((-2, -1), keepdims=True)
    return (x - m) / jnp.sqrt(v + EPS) * w + b


def _linear_attn(x, Wq, Wk, Wv, Wo, bo):
    b, n, d = x.shape
    dh = d // H

    def heads(t):
        return t.reshape(b, n, H, dh).transpose(0, 2, 1, 3)

    q, k, v = heads(x @ Wq), heads(x @ Wk), heads(x @ Wv)
    q = jax.nn.softmax(q, axis=-1) * (dh ** -0.5)
    k = jax.nn.softmax(k, axis=-2)
    ctx = jnp.einsum("bhnd,bhne->bhde", k, v)
    out = jnp.einsum("bhnd,bhde->bhne", q, ctx)
    out = out.transpose(0, 2, 1, 3).reshape(b, n, d)
    return out @ Wo + bo


def _lf_block(x, p):
    (ln1_w, ln1_b, Wq, Wk, Wv, Wo, bo, ln2_w, ln2_b, W1, b1, W2, b2, Wd, bd) = p
    x = x + _linear_attn(_ln(x, ln1_w, ln1_b), Wq, Wk, Wv, Wo, bo)
    h = _ln(x, ln2_w, ln2_b)
    h = jax.nn.gelu(h @ W1 + b1, approximate=False) @ W2 + b2
    x = x + h
    return x @ Wd + bd


def _compute_one(x, *params):
    """x: [1, D, T, NF] for one batch element (the pmap-local shard)."""
    d = dict(zip(_PARAM_ORDER, params))
    fp = (d["f_ln1_w"], d["f_ln1_b"], d["f_Wq"], d["f_Wk"], d["f_Wv"], d["f_Wo"],
          d["f_bo"], d["f_ln2_w"], d["f_ln2_b"], d["f_W1"], d["f_b1"], d["f_W2"],
          d["f_b2"], d["f_Wd"], d["f_bd"])
    tp = (d["t_ln1_w"], d["t_ln1_b"], d["t_Wq"], d["t_Wk"], d["t_Wv"], d["t_Wo"],
          d["t_bo"], d["t_ln2_w"], d["t_ln2_b"], d["t_W1"], d["t_b1"], d["t_W2"],
          d["t_b2"], d["t_Wd"], d["t_bd"])
    b, dd, t, f = x.shape
    x = x.transpose(0, 2, 3, 1)                     # [b, T, F, D]
    res = x
    h = _ln2d(x, d["norm_w"], d["norm_b"]).reshape(b * t, f, dd)
    x = _lf_block(h, fp).reshape(b, t, f, dd) + res
    res = x
    h = _ln2d(x, d["norm_w"], d["norm_b"]).transpose(0, 2, 1, 3).reshape(b * f, t, dd)
    h = _lf_block(h, tp).reshape(b, f, t, dd).transpose(0, 2, 1, 3)
    x = res + h
    return x.transpose(0, 3, 1, 2)                  # [b, D, T, F]


_COMPILED = None


def _get_compiled():
    global _COMPILED
    if _COMPILED is None:
        devs = jax.devices()[:8]
        _COMPILED = jax.pmap(
            _compute_one,
            in_axes=(0,) + (None,) * len(_PARAM_ORDER),
            devices=devs,
        )
    return _COMPILED


def kernel(**inputs) -> np.ndarray:
    x = np.asarray(inputs["x"], dtype=np.float32)
    assert x.shape == (B, D, T, NF), x.shape
    params = [np.asarray(inputs[k], dtype=np.float32) for k in _PARAM_ORDER]
    fn = _get_compiled()
    # [8, 1, D, T, NF] so each device sees a [1, D, T, NF] shard
    xs = x.reshape(B, 1, D, T, NF)
    out = fn(xs, *params)                           # [8, 1, D, T, NF]
    out = np.asarray(out, dtype=np.float32).reshape(B, D, T, NF)
    return out


# revision 2
# speedup vs baseline: 27.3710x; 27.3710x over previous
"""DualPathLF kernel for 8 Trainium2 NeuronCores.

Sharding: data-parallel over B (B=8 == n_cores). Each core gets one batch
element; both dual-path attentions reshape batch into B*T / B*F so the whole
computation is embarrassingly parallel over B. Params are replicated.

Device program notes:
- matmuls/einsums run in bf16 with fp32 accumulation (PE runs bf16 at 4x the
  fp32 rate); LayerNorms, softmaxes and residuals stay fp32.
- the linear-attention context k^T v is computed cross-head as one [64,64]
  batched matmul then masked to the block-diagonal head blocks, replacing
  4x as many tiny [16,16] batched matmuls.

Self-contained: hardcodes shapes B=8, D=64, T=2000, NF=32, H=4.
"""

import numpy as np
import jax
import jax.numpy as jnp

B, D, T, NF, H = 8, 64, 2000, 32, 4
EPS = 1e-5
DH = D // H

_PARAM_ORDER = [
    "norm_w", "norm_b",
    "f_ln1_w", "f_ln1_b", "f_Wq", "f_Wk", "f_Wv", "f_Wo", "f_bo",
    "f_ln2_w", "f_ln2_b", "f_W1", "f_b1", "f_W2", "f_b2", "f_Wd", "f_bd",
    "t_ln1_w", "t_ln1_b", "t_Wq", "t_Wk", "t_Wv", "t_Wo", "t_bo",
    "t_ln2_w", "t_ln2_b", "t_W1", "t_b1", "t_W2", "t_b2", "t_Wd", "t_bd",
]

# Block-diagonal head mask for the cross-head context matmul
_MASK = np.kron(np.eye(H, dtype=np.float32), np.ones((DH, DH), np.float32))


def _mm(a, b):
    return jnp.matmul(a.astype(jnp.bfloat16), b.astype(jnp.bfloat16),
                      preferred_element_type=jnp.float32)


def _ein(spec, a, b):
    return jnp.einsum(spec, a.astype(jnp.bfloat16), b.astype(jnp.bfloat16),
                      preferred_element_type=jnp.float32)


def _ln(x, w, b):
    m = x.mean(-1, keepdims=True)
    v = ((x - m) ** 2).mean(-1, keepdims=True)
    return (x - m) / jnp.sqrt(v + EPS) * w + b


def _ln2d(x, w, b):
    m = x.mean((-2, -1), keepdims=True)
    v = ((x - m) ** 2).mean((-2, -1), keepdims=True)
    return (x - m) / jnp.sqrt(v + EPS) * w + b


def _linear_attn(x, Wq, Wk, Wv, Wo, bo):
    b, n, d = x.shape
    q = _mm(x, Wq).reshape(b, n, H, DH)
    k = _mm(x, Wk).reshape(b, n, H, DH)
    v = _mm(x, Wv)                                    # [b, n, D]
    q = jax.nn.softmax(q, axis=-1).reshape(b, n, d) * (DH ** -0.5)
    k = jax.nn.softmax(k, axis=1).reshape(b, n, d)    # over n, per (h, dh)
    ctx = _ein("bnd,bne->bde", k, v) * _MASK          # [b, D, D] block-diag
    out = _ein("bnd,bde->bne", q, ctx)
    return _mm(out, Wo) + bo


def _lf_block(x, p):
    (ln1_w, ln1_b, Wq, Wk, Wv, Wo, bo, ln2_w, ln2_b, W1, b1, W2, b2, Wd, bd) = p
    x = x + _linear_attn(_ln(x, ln1_w, ln1_b), Wq, Wk, Wv, Wo, bo)
    h = _ln(x, ln2_w, ln2_b)
    h = _mm(jax.nn.gelu(_mm(h, W1) + b1, approximate=False), W2) + b2
    x = x + h
    return _mm(x, Wd) + bd


def _compute_one(x, *params):
    """x: [1, D, T, NF] — the pmap-local shard (one batch element)."""
    d = dict(zip(_PARAM_ORDER, params))
    keys = ("ln1_w", "ln1_b", "Wq", "Wk", "Wv", "Wo", "bo",
            "ln2_w", "ln2_b", "W1", "b1", "W2", "b2", "Wd", "bd")
    fp = tuple(d["f_" + k] for k in keys)
    tp = tuple(d["t_" + k] for k in keys)
    b, dd, t, f = x.shape
    x = x.transpose(0, 2, 3, 1)                       # [b, T, F, D]
    res = x
    h = _ln2d(x, d["norm_w"], d["norm_b"]).reshape(b * t, f, dd)
    x = _lf_block(h, fp).reshape(b, t, f, dd) + res
    res = x
    h = _ln2d(x, d["norm_w"], d["norm_b"]).transpose(0, 2, 1, 3).reshape(b * f, t, dd)
    h = _lf_block(h, tp).reshape(b, f, t, dd).transpose(0, 2, 1, 3)
    x = res + h
    return x.transpose(0, 3, 1, 2)                    # [b, D, T, F]


_COMPILED = None


def _get_compiled():
    global _COMPILED
    if _COMPILED is None:
        devs = jax.devices()[:8]
        _COMPILED = jax.pmap(
            _compute_one,
            in_axes=(0,) + (None,) * len(_PARAM_ORDER),
            devices=devs,
        )
    return _COMPILED


def kernel(**inputs) -> np.ndarray:
    x = np.asarray(inputs["x"], dtype=np.float32)
    assert x.shape == (B, D, T, NF), x.shape
    params = [np.asarray(inputs[k], dtype=np.float32) for k in _PARAM_ORDER]
    fn = _get_compiled()
    # [8, 1, D, T, NF] so each device sees a [1, D, T, NF] shard
    xs = x.reshape(B, 1, D, T, NF)
    out = fn(xs, *params)                             # [8, 1, D, T, NF]
    out = np.asarray(out, dtype=np.float32).reshape(B, D, T, NF)
    return out
